# revision 4
# baseline (speedup 1.0000x reference)
"""Trainium2 Bass kernel for nn_Decoder (teacher-forced GRU decoder).

Problem: T=1024, B=256, H=512, O=64.
  xs = [0, target[:-1]]; h0 = code_vec[0]
  per step: gi = x@w_ih.T + b_ih ; gh = h@w_hh.T + b_hh
            r = sig(i_r+h_r); z = sig(i_z+h_z); n = tanh(i_n + r*h_n)
            h' = (1-z)*n + z*h ; y = h'@w_out.T + b_out
  returns (outputs [T,B,O], h_last [1,B,H])

Strategy: data-parallel over batch, B=32 per core on 8 cores. Per-core:
  - state kept two ways: hb (B-major [32,512] fp32, for the gate math) and
    hT (H-major [128,4,32] fp32r, the matmul stationary operand), produced
    each step by DVE 32x32 block-transpose + 4 partition-rebase SBUF DMAs.
  - all matmuls in fp32r (full PE rate, ~1e-3 rounding).
  - biases ride a ones-row appended to x (xT has 65 rows; waug carries
    [w_ih | biases] column blocks for r/z/gn/hn-bias/y-bias).
  - all host-side transposes (weights, x, h0) are done in numpy here.

kernel(**inputs) -> (outputs, h_last) matching reference().
"""

import functools
import time
from contextlib import ExitStack

import numpy as np

import concourse.bass as bass
import concourse.mybir as mybir
import concourse.tile as tile

T, B, H, O = 1024, 256, 512, 64
NCORES = 8
BC = B // NCORES  # 32 batch per core
KCH = H // 128  # 4 k-chunks
XROWS = O + 1  # 65: x plus ones row
XCHUNK = 128  # timesteps per x prefetch chunk

F32 = mybir.dt.float32
F32R = mybir.dt.float32r
AF = mybir.ActivationFunctionType
ALU = mybir.AluOpType


def _split_excess_waits(nc, max_waits=1):
    """This walrus build accepts only one sync-wait per instruction; spill
    extras onto same-engine no-ops placed just before."""
    ctr = 0
    for f in nc.m.functions:
        for b in f.blocks:
            insts = list(b.instructions)
            if not any(
                i.sync_info
                and i.sync_info.on_wait
                and len(i.sync_info.on_wait) > max_waits
                for i in insts
            ):
                continue
            new_list = []
            for inst in insts:
                si = inst.sync_info
                if si and si.on_wait and len(si.on_wait) > max_waits:
                    waits = list(si.on_wait)
                    spill, keep = waits[:-max_waits], waits[-max_waits:]
                    while spill:
                        chunk, spill = spill[:max_waits], spill[max_waits:]
                        ctr += 1
                        new_list.append(
                            mybir.InstNoOp(
                                name=f"waitnop-{ctr}-{inst.name}",
                                engine=inst.engine,
                                sync_info=mybir.SyncInfo(on_wait=chunk, on_update=[]),
                            )
                        )
                    inst.sync_info = mybir.SyncInfo(
                        on_wait=keep, on_update=list(si.on_update or [])
                    )
                new_list.append(inst)
            b.instructions = new_list
    return ctr


def build_gru(nsteps):
    nc = bass.Bass("TRN2", target_bir_lowering=False, debug=False, num_devices=NCORES)

    xT = nc.dram_tensor("xT", [XROWS, nsteps, BC], F32R, kind="ExternalInput").ap()
    whhT = nc.dram_tensor("whhT", [128, KCH, 3 * H], F32R, kind="ExternalInput").ap()
    waug = nc.dram_tensor("waug", [XROWS, 3 * H + H + O], F32R, kind="ExternalInput").ap()
    woutT = nc.dram_tensor("woutT", [128, KCH, O], F32R, kind="ExternalInput").ap()
    h0T = nc.dram_tensor("h0T", [128, KCH, BC], F32R, kind="ExternalInput").ap()
    h0b = nc.dram_tensor("h0b", [BC, H], F32R, kind="ExternalInput").ap()
    yD = nc.dram_tensor("y", [nsteps, BC, O], F32, kind="ExternalOutput").ap()
    hlastD = nc.dram_tensor("hlast", [BC, H], F32R, kind="ExternalOutput").ap()

    # waug column blocks
    CR, CZ, CN, CHB, CY = (
        slice(0, H),
        slice(H, 2 * H),
        slice(2 * H, 3 * H),
        slice(3 * H, 4 * H),
        slice(4 * H, 4 * H + O),
    )

    with tile.TileContext(nc) as tc, ExitStack() as ctx:
        const = ctx.enter_context(tc.tile_pool(name="const", bufs=1))
        xpool = ctx.enter_context(tc.tile_pool(name="xc", bufs=2))
        state = ctx.enter_context(tc.tile_pool(name="state", bufs=3))
        gates = ctx.enter_context(tc.tile_pool(name="gates", bufs=2))
        ypool = ctx.enter_context(tc.tile_pool(name="yp", bufs=3))
        psum = ctx.enter_context(tc.tile_pool(name="ps", bufs=1, space="PSUM"))
        psy = ctx.enter_context(tc.tile_pool(name="psy", bufs=2, space="PSUM"))

        whh_sb = const.tile([128, KCH, 3 * H], F32R)
        nc.sync.dma_start(out=whh_sb, in_=whhT)
        waug_sb = const.tile([XROWS, 4 * H + O], F32R)
        nc.sync.dma_start(out=waug_sb, in_=waug)
        wout_sb = const.tile([128, KCH, O], F32R)
        nc.sync.dma_start(out=wout_sb, in_=woutT)

        # state buffers (persistent across the step loop): allocate from
        # pools per-step so Tile double-buffers them.
        hT_prev = state.tile([128, KCH, BC], F32R, tag="hT")
        nc.sync.dma_start(out=hT_prev, in_=h0T)
        hb_prev = state.tile([BC, H], F32R, tag="hb")
        nc.sync.dma_start(out=hb_prev, in_=h0b)

        nchunks = (nsteps + XCHUNK - 1) // XCHUNK
        x_chunks = []
        xc = xpool.tile([XROWS, XCHUNK, BC], F32R, tag="xc")
        nc.sync.dma_start(out=xc[:, : min(XCHUNK, nsteps), :], in_=xT[:, :XCHUNK, :])
        x_chunks.append(xc)

        def emit_y_mms(hT, xt):
            py = psy.tile([BC, O], F32, tag="py")
            nc.tensor.matmul(py, xt, waug_sb[:, CY], start=True, stop=False)
            for k in range(KCH):
                nc.tensor.matmul(
                    py,
                    hT[:, k, :],
                    wout_sb[:, k, :],
                    start=False,
                    stop=(k == KCH - 1),
                )
            return py

        for t in range(nsteps):
            ci = t // XCHUNK
            if t % XCHUNK == 0 and ci + 1 < nchunks:
                # prefetch next chunk into the other buffer
                c0 = (ci + 1) * XCHUNK
                c1 = min(c0 + XCHUNK, nsteps)
                xc = xpool.tile([XROWS, XCHUNK, BC], F32R, tag="xc")
                nc.sync.dma_start(out=xc[:, : c1 - c0, :], in_=xT[:, c0:c1, :])
                x_chunks.append(xc)
            xt = x_chunks[ci][:, t % XCHUNK, :]

            # ---- y output for the previous state (h_t), t >= 1
            if t >= 1:
                py = emit_y_mms(hT_prev, xt)
                y_sb = ypool.tile([BC, O], F32, tag="ysb")
                nc.scalar.activation(y_sb, py, AF.Copy)
                nc.sync.dma_start(out=yD[t - 1], in_=y_sb)

            # ---- z gate (first: its products are needed late but start the
            # chain's cheap precomputations early)
            pz = psum.tile([BC, H], F32, tag="pz")
            nc.tensor.matmul(pz, xt, waug_sb[:, CZ], start=True, stop=False)
            for k in range(KCH):
                nc.tensor.matmul(
                    pz, hT_prev[:, k, :], whh_sb[:, k, CZ], start=False,
                    stop=(k == KCH - 1),
                )
            z = gates.tile([BC, H], F32, tag="z")
            nc.scalar.activation(z, pz, AF.Sigmoid)
            zc = gates.tile([BC, H], F32, tag="zc")  # 1 - z
            nc.vector.tensor_scalar(zc, z, -1.0, 1.0, ALU.mult, ALU.add)
            v = gates.tile([BC, H], F32, tag="v")  # z * h
            nc.vector.tensor_mul(v, z, hb_prev.bitcast(F32))

            # ---- r gate
            pr = psum.tile([BC, H], F32, tag="pr")
            nc.tensor.matmul(pr, xt, waug_sb[:, CR], start=True, stop=False)
            for k in range(KCH):
                nc.tensor.matmul(
                    pr, hT_prev[:, k, :], whh_sb[:, k, CR], start=False,
                    stop=(k == KCH - 1),
                )
            r = gates.tile([BC, H], F32, tag="r")
            nc.scalar.activation(r, pr, AF.Sigmoid)

            # ---- n gate: gn = x part (+b_ih_n); hn = h part (+b_hh_n)
            pgn = psum.tile([BC, H], F32, tag="pgn")
            nc.tensor.matmul(pgn, xt, waug_sb[:, CN], start=True, stop=True)
            phn = psum.tile([BC, H], F32, tag="phn")
            nc.tensor.matmul(phn, xt, waug_sb[:, CHB], start=True, stop=False)
            for k in range(KCH):
                nc.tensor.matmul(
                    phn, hT_prev[:, k, :], whh_sb[:, k, CN], start=False,
                    stop=(k == KCH - 1),
                )

            t1 = gates.tile([BC, H], F32, tag="t1")
            nc.vector.tensor_mul(t1, r, phn)
            t2 = gates.tile([BC, H], F32, tag="t2")
            nc.vector.tensor_add(t2, t1, pgn)
            n = gates.tile([BC, H], F32, tag="n")
            nc.scalar.activation(n, t2, AF.Tanh)

            u = gates.tile([BC, H], F32, tag="u")  # (1-z)*n
            nc.vector.tensor_mul(u, zc, n)
            hb_new = state.tile([BC, H], F32R, tag="hb")
            nc.vector.tensor_add(hb_new, u, v)

            # ---- transpose h' -> hT (32x32 blocks then partition rebase)
            hTb = gates.tile([BC, H // BC, BC], F32R, tag="hTb")
            nc.vector.transpose(
                hTb.rearrange("p a b -> p (a b)").bitcast(F32), hb_new.bitcast(F32)
            )
            hT_new = state.tile([128, KCH, BC], F32R, tag="hT")
            hTb_v = hTb.rearrange("i (k jj) b -> i jj k b", jj=4)
            for jj in range(4):
                nc.sync.dma_start(
                    out=hT_new[bass.ts(jj, 32), :, :], in_=hTb_v[:, jj]
                )

            hT_prev, hb_prev = hT_new, hb_new

        # final y for h_{nsteps} and h_last
        py = emit_y_mms(hT_prev, x_chunks[-1][:, (nsteps - 1) % XCHUNK, :])
        y_sb = ypool.tile([BC, O], F32, tag="ysb")
        nc.scalar.activation(y_sb, py, AF.Copy)
        nc.sync.dma_start(out=yD[nsteps - 1], in_=y_sb)
        nc.sync.dma_start(out=hlastD, in_=hb_prev)

    nsplit = _split_excess_waits(nc)
    return nc, nsplit


def prep_core_inputs(code_vec, target, w_ih, w_hh, b_ih, b_hh, w_out, b_out, nsteps):
    """Host-side shard + transpose prep. Returns list of per-core in_maps."""
    f32 = np.float32
    xs = np.concatenate(
        [np.zeros((1, B, O), f32), np.asarray(target[: nsteps - 1], f32)], axis=0
    )  # [nsteps, B, O]
    whhT_h = np.ascontiguousarray(
        np.asarray(w_hh, f32).T.reshape(KCH, 128, 3 * H).transpose(1, 0, 2)
    )  # [128, KCH, 3H]
    woutT_h = np.ascontiguousarray(
        np.asarray(w_out, f32).T.reshape(KCH, 128, O).transpose(1, 0, 2)
    )
    wihT = np.asarray(w_ih, f32).T  # [O, 3H]
    b = np.asarray(b_ih, f32) + np.asarray(b_hh, f32)
    waug_h = np.zeros((XROWS, 4 * H + O), f32)
    waug_h[:O, 0 : 3 * H] = wihT
    waug_h[:O, 3 * H : 4 * H] = 0.0
    waug_h[O, 0:H] = b[0:H]
    waug_h[O, H : 2 * H] = b[H : 2 * H]
    waug_h[O, 2 * H : 3 * H] = np.asarray(b_ih, f32)[2 * H : 3 * H]
    waug_h[O, 3 * H : 4 * H] = np.asarray(b_hh, f32)[2 * H : 3 * H]
    waug_h[O, 4 * H :] = np.asarray(b_out, f32)

    h0 = np.asarray(code_vec, f32)[0]  # [B, H]
    in_maps = []
    for c in range(NCORES):
        bsl = slice(c * BC, (c + 1) * BC)
        xTc = np.empty((XROWS, nsteps, BC), f32)
        xTc[:O] = xs[:, bsl, :].transpose(2, 0, 1)
        xTc[O] = 1.0
        h0c = np.ascontiguousarray(h0[bsl])  # [BC, H]
        h0Tc = np.ascontiguousarray(
            h0c.T.reshape(KCH, 128, BC).transpose(1, 0, 2)
        )
        in_maps.append(
            {
                "xT": xTc,
                "whhT": whhT_h,
                "waug": waug_h,
                "woutT": woutT_h,
                "h0T": h0Tc,
                "h0b": h0c,
            }
        )
    return in_maps


@functools.lru_cache(maxsize=2)
def _get_built(nsteps):
    nc, nsplit = build_gru(nsteps)
    return nc


def run_gru(inputs, nsteps=T):
    import concourse.bass_utils as bass_utils

    nc = _get_built(nsteps)
    in_maps = prep_core_inputs(nsteps=nsteps, **inputs)
    res = bass_utils.run_bass_kernel_spmd(nc, in_maps, core_ids=list(range(NCORES)))
    y = np.concatenate([r["y"] for r in res.results], axis=1)  # [nsteps, B, O]
    hlast = np.concatenate([r["hlast"] for r in res.results], axis=0)  # [B, H]
    return y, hlast[None]


def kernel(**inputs):
    return run_gru(inputs, nsteps=T)


if __name__ == "__main__":
    # quick self-test against a numpy reference at small T
    nsteps = int(__import__("sys").argv[1]) if len(__import__("sys").argv) > 1 else 64

    rng = np.random.default_rng(0)
    s = 1.0 / np.sqrt(H)
    inputs = {
        "code_vec": rng.standard_normal((1, B, H)).astype(np.float32),
        "target": rng.standard_normal((T, B, O)).astype(np.float32),
        "w_ih": (rng.standard_normal((3 * H, O)) * s).astype(np.float32),
        "w_hh": (rng.standard_normal((3 * H, H)) * s).astype(np.float32),
        "b_ih": (rng.standard_normal(3 * H) * s).astype(np.float32),
        "b_hh": (rng.standard_normal(3 * H) * s).astype(np.float32),
        "w_out": (rng.standard_normal((O, H)) * s).astype(np.float32),
        "b_out": (rng.standard_normal(O) * s).astype(np.float32),
    }

    def gru_numpy(code_vec, target, w_ih, w_hh, b_ih, b_hh, w_out, b_out, nsteps):
        dt = np.float64
        xs = np.concatenate(
            [np.zeros((1, B, O), dt), np.asarray(target[: nsteps - 1], dt)], axis=0
        )
        h = np.asarray(code_vec, dt)[0]
        ys = []
        for tt in range(nsteps):
            gi = xs[tt] @ np.asarray(w_ih, dt).T + np.asarray(b_ih, dt)
            gh = h @ np.asarray(w_hh, dt).T + np.asarray(b_hh, dt)
            i_r, i_z, i_n = np.split(gi, 3, axis=-1)
            h_r, h_z, h_n = np.split(gh, 3, axis=-1)
            rr = 1 / (1 + np.exp(-(i_r + h_r)))
            zz = 1 / (1 + np.exp(-(i_z + h_z)))
            nn_ = np.tanh(i_n + rr * h_n)
            h = (1 - zz) * nn_ + zz * h
            ys.append(h @ np.asarray(w_out, dt).T + np.asarray(b_out, dt))
        return np.stack(ys), h[None]

    t0 = time.time()
    y, hl = run_gru(inputs, nsteps=nsteps)
    t1 = time.time()
    print(f"run_gru({nsteps}) wall: {t1-t0:.1f}s")
    yr, hr = gru_numpy(nsteps=nsteps, **inputs)
    yerr = np.abs(y - yr)
    herr = np.abs(hl - hr)
    print(
        f"y rel: {np.linalg.norm(y - yr) / np.linalg.norm(yr):.3e} "
        f"absmax {yerr.max():.3e} (scale {np.abs(yr).max():.2f})"
    )
    print(
        f"h rel: {np.linalg.norm(hl - hr) / np.linalg.norm(hr):.3e} "
        f"absmax {herr.max():.3e}"
    )
